# revision 34
# baseline (speedup 1.0000x reference)
"""ContextBasedLinear Trainium2 kernel.

Computes out = mu * x + gamma * sum(x, axis=1, keepdims=True) for
x: [64, 1024, 512] f32, mu/gamma: [1] f32.

Sharding: data-parallel on the batch dim across 8 NeuronCores;
mu/gamma replicated. No cross-core comms needed. The batch split is
UNEVEN: on this box the even-numbered devices are the only ones that
ever show a hot SDMA engine (~+18% DMA time; observed as DMA_0/DMA_15
across many runs, never on odd devices), so odd devices take 9 batches
and even devices 7 (64 total). Each batch count is its own Bass
program, launched on its device group with an explicit jax device list
(bass_utils' launcher always uses devices [0..k), which cannot target
the odd devices) — the HW exec metric is the max per-core useful span,
and this split equalizes the spans.

Per-core program (x_c: [b_per, 1024, 512]):
  Each batch's [1024, 512] lives in SBUF as [128, 4096]: partition p
  holds set rows 8p..8p+7 (16 KB contiguous per partition), processed
  in two half-tiles [128, 2048] for pipelining.
  - colsum: PE matmuls with ones[128,1] stationary reduce the
    partition dim of each 512-wide r-slice, accumulating all 8 slices
    into one PSUM row psum_s[1, 512]. Inputs viewed as float32r:
    single-pass fp32 matmul (1 cyc/row at N=512 vs 4 for the fp32
    LOW_HIGH split) — PE busy drops ~4x; the reduced-precision
    accumulate costs ~1e-3 rel err vs the 2e-2 gate. The BIR verifier
    wants FP32r operands produced "rounded": the x loads use f32r APs
    on both sides, and the ones lhsT is DMA-fed from a host input
    (walrus rejects f32r memset).
  - psum_b[128,512] = (gamma ones)[1,128].T @ s[1,512]: rank-1 matmul
    broadcasts gamma * colsum to every partition (plain fp32: only
    ~0.9us/batch of PE).
  - out = (x * mu) + psum_b in ONE fused DVE scalar_tensor_tensor pass
    per half, with psum_b read through a step-0 broadcast AP. The DVE
    writes fp16: stores move half the bytes (HBM-bound kernel; fp16
    rounding is ~5e-4 rel), upcast to f32 on the host.
  - Ring assignment: batch 0 loads BOTH halves on the sync HWDGE ring
    so its data completes first (~14 us); its colsum/bcast/first STT —
    and therefore the store stream — start early, which shrinks the
    end-of-kernel store drain. b1/b2 odd halves ride the (load-idle)
    ACT ring behind the tiny const DMAs; stores go on ACT except the
    last two batches' (sync), mirroring the drain.
"""

import numpy as np

import concourse.bacc as bacc
import concourse.mybir as mybir
import concourse.tile as tile

N_CORES = 8
B_FULL = 64
# batches per device (sum = 64): odd devices 9, even (hot-DMA-prone) 7
CORE_BATCHES = [7, 9, 7, 9, 7, 9, 7, 9]
OFFSETS = np.concatenate([[0], np.cumsum(CORE_BATCHES)])
GROUPS = []
for _bp in sorted(set(CORE_BATCHES), reverse=True):
    GROUPS.append((_bp, [c for c in range(N_CORES) if CORE_BATCHES[c] == _bp]))

N_SET = 1024
D = 512
P = 128
R = N_SET // P  # 8 set-rows per partition
F = R * D  # 4096 free elems per partition
H = 2  # half-tiles per batch
RH = R // H  # 4 r-slices per half
FH = F // H  # 2048 free elems per half

_cache = {}


def build_nc(b_per):
    if b_per in _cache:
        return _cache[b_per]
    f32 = mybir.dt.float32
    f32r = mybir.dt.float32r
    f16 = mybir.dt.float16
    nc = bacc.Bacc(
        "TRN2", target_bir_lowering=False, debug=False, num_devices=N_CORES
    )
    x_d = nc.dram_tensor("x", [b_per, N_SET, D], f32, kind="ExternalInput").ap()
    mu_d = nc.dram_tensor("mu", [1], f32, kind="ExternalInput").ap()
    gamma_d = nc.dram_tensor("gamma", [1], f32, kind="ExternalInput").ap()
    # host-fed ones: walrus can't memset f32r, but a DMA with f32r APs is
    # accepted as an FP32r-rounded producer for the colsum lhsT
    ones_d = nc.dram_tensor("ones", [P], f32, kind="ExternalInput").ap()
    out_d = nc.dram_tensor("out", [b_per, N_SET, D], f16, kind="ExternalOutput").ap()

    with tile.TileContext(nc) as tc:
        with (
            tc.tile_pool(name="consts", bufs=1) as consts,
            tc.tile_pool(name="xp", bufs=12) as xp,
            tc.tile_pool(name="op", bufs=9) as op,
            tc.tile_pool(name="sp", bufs=2) as sp,
            tc.tile_pool(name="ps", bufs=2, space="PSUM") as ps,
            tc.tile_pool(name="pb", bufs=2, space="PSUM") as pb,
        ):
            # ---- constants ----
            # The tiny const DMAs lead the ACT ring (ahead of the b1/b2 odd
            # halves): HWDGE delivers them by ~10.5 us. The SWDGE (gpsimd)
            # ring was tried and takes ~10 us to deliver 4 bytes — it gated
            # the first colsum until ~19 us.
            ones_col = consts.tile([P, 1], f32)  # colsum lhsT (K=128, M=1)
            nc.scalar.dma_start(
                ones_col[:].bitcast(f32r), ones_d[:, None].bitcast(f32r)
            )
            ones_row = consts.tile([1, P], f32)
            nc.vector.memset(ones_row, 1.0)
            mu_sb = consts.tile([1, 1], f32)
            nc.scalar.dma_start(mu_sb, mu_d[None, :])
            gamma_sb = consts.tile([1, 1], f32)
            nc.scalar.dma_start(gamma_sb, gamma_d[None, :])
            # gamma_row[1,128] = gamma * ones (runtime scalar from SBUF)
            gamma_row = consts.tile([1, P], f32)
            nc.vector.tensor_scalar_mul(gamma_row, ones_row, gamma_sb[:])
            # mu replicated to all 128 partitions via rank-1 matmul
            psum_mu = ps.tile([P, 1], f32, tag="psmu")
            nc.tensor.matmul(
                psum_mu, lhsT=ones_row[:], rhs=mu_sb[:], start=True, stop=True
            )
            mu_col = consts.tile([P, 1], f32)
            nc.vector.tensor_copy(mu_col, psum_mu)

            # ---- per-batch pipeline ----
            for b in range(b_per):
                x_view = x_d[b].rearrange("(p r) d -> p (r d)", p=P)
                o_view = out_d[b].rearrange("(p r) d -> p (r d)", p=P)

                xts = []
                for h in range(H):
                    xt = xp.tile([P, FH], f32, tag="xt")
                    # Batch 0 loads BOTH halves on the sync ring so its data
                    # completes first; b1/b2 odd halves ride the ACT ring to
                    # keep two HWDGE queue rows busy during the ramp (~390
                    # GB/s one row, ~422 with two); no HOL risk for those.
                    eng = nc.scalar if (b in (1, 2) and h == 1) else nc.sync
                    eng.dma_start(
                        xt[:].bitcast(f32r),
                        x_view[:, h * FH : (h + 1) * FH].bitcast(f32r),
                    )
                    xts.append(xt)

                # colsum over all 1024 set rows -> psum_s[1, 512]
                psum_s = ps.tile([1, D], f32, tag="pss")
                for h in range(H):
                    for j in range(RH):
                        nc.tensor.matmul(
                            psum_s,
                            lhsT=ones_col[:].bitcast(f32r),
                            rhs=xts[h][:, j * D : (j + 1) * D].bitcast(f32r),
                            start=(h == 0 and j == 0),
                            stop=(h == H - 1 and j == RH - 1),
                        )
                s_sb = sp.tile([1, D], f32, tag="ssb")
                nc.scalar.copy(s_sb, psum_s)

                # broadcast gamma*colsum to [128, 512] via rank-1 matmul
                psum_b = pb.tile([P, D], f32, tag="psb")
                nc.tensor.matmul(
                    psum_b, lhsT=gamma_row[:], rhs=s_sb[:], start=True, stop=True
                )

                # fused: out = (x * mu) + bcast   (single DVE pass per chunk).
                # The last batch runs quarter-size chunks so the kernel tail
                # (final STT + final store) is half as long.
                nq = 2 if b == b_per - 1 else 1
                fq = FH // nq
                rq = RH // nq
                for h in range(H):
                    for q in range(nq):
                        ot = op.tile([P, fq], f16, tag="ot")
                        nc.vector.scalar_tensor_tensor(
                            out=ot[:].rearrange("p (r d) -> p r d", r=rq),
                            in0=xts[h][:, q * fq : (q + 1) * fq].rearrange(
                                "p (r d) -> p r d", r=rq
                            ),
                            scalar=mu_col[:],
                            in1=psum_b[:, None, :].broadcast_to([P, rq, D]),
                            op0=mybir.AluOpType.mult,
                            op1=mybir.AluOpType.add,
                        )
                        # Mirror trick for the store-only drain: all loads
                        # have issued by the time the last batches store, so
                        # the sync row is free to carry half the tail stores.
                        seng = nc.sync if b >= b_per - 2 else nc.scalar
                        seng.dma_start(
                            o_view[:, h * FH + q * fq : h * FH + (q + 1) * fq], ot
                        )

    nc.compile()
    _cache[b_per] = nc
    return nc


def run_pinned(nc, in_maps, device_ids):
    """run_bass_via_pjrt with an explicit device list: lands a k-core
    launch on arbitrary physical cores (the stock launcher always takes
    jax.devices()[:k])."""
    import jax
    from jax.sharding import Mesh, PartitionSpec
    from jax.experimental.shard_map import shard_map

    from concourse import bass2jax
    from concourse.bass2jax import _bass_exec_p, partition_id_tensor

    bass2jax.install_neuronx_cc_hook()

    partition_name = nc.partition_id_tensor.name if nc.partition_id_tensor else None

    in_names, out_names, out_avals, zero_outs = [], [], [], []
    for alloc in nc.m.functions[0].allocations:
        if not isinstance(alloc, mybir.MemoryLocationSet):
            continue
        name = alloc.memorylocations[0].name
        if alloc.kind == "ExternalInput":
            if name != partition_name:
                in_names.append(name)
        elif alloc.kind == "ExternalOutput":
            shape = tuple(alloc.tensor_shape)
            dtype = mybir.dt.np(alloc.dtype)
            out_avals.append(jax.core.ShapedArray(shape, dtype))
            out_names.append(name)
            zero_outs.append(np.zeros(shape, dtype))
    n_params = len(in_names)
    n_outs = len(out_avals)
    in_names.extend(out_names)
    if partition_name is not None:
        in_names.append(partition_name)

    donate = tuple(range(n_params, n_params + n_outs))

    def _body(*args):
        operands = list(args)
        if partition_name is not None:
            operands.append(partition_id_tensor())
        outs = _bass_exec_p.bind(
            *operands,
            out_avals=tuple(out_avals),
            in_names=tuple(in_names),
            out_names=tuple(out_names),
            lowering_input_output_aliases=(),
            sim_require_finite=True,
            sim_require_nnan=True,
            nc=nc,
        )
        return tuple(outs)

    n_cores = len(device_ids)
    devices = [jax.devices()[i] for i in device_ids]
    mesh = Mesh(np.asarray(devices), ("core",))
    in_specs = (PartitionSpec("core"),) * (n_params + n_outs)
    out_specs = (PartitionSpec("core"),) * len(out_names)
    sharded = jax.jit(
        shard_map(
            _body, mesh=mesh, in_specs=in_specs, out_specs=out_specs, check_rep=False
        ),
        donate_argnums=donate,
        keep_unused=True,
    )
    per_core = [[np.asarray(m[name]) for name in in_names[:n_params]] for m in in_maps]
    concat_in = [
        np.concatenate([per_core[c][i] for c in range(n_cores)], axis=0)
        for i in range(n_params)
    ]
    concat_zeros = [
        np.zeros((n_cores * z.shape[0], *z.shape[1:]), z.dtype) for z in zero_outs
    ]
    out_arrs = sharded(*concat_in, *concat_zeros)
    return [
        {
            name: np.asarray(out_arrs[i]).reshape(n_cores, *out_avals[i].shape)[c]
            for i, name in enumerate(out_names)
        }
        for c in range(n_cores)
    ]


def group_in_maps(x, mu, gamma, b_per, cores):
    ones = np.ones([P], dtype=np.float32)
    return [
        {
            "x": x[OFFSETS[c] : OFFSETS[c] + b_per],
            "mu": mu,
            "gamma": gamma,
            "ones": ones,
        }
        for c in cores
    ]


def kernel(x, mu, gamma):
    x = np.ascontiguousarray(x, dtype=np.float32)
    mu = np.ascontiguousarray(mu, dtype=np.float32)
    gamma = np.ascontiguousarray(gamma, dtype=np.float32)
    out = np.empty((B_FULL, N_SET, D), dtype=np.float32)
    for b_per, cores in GROUPS:
        nc = build_nc(b_per)
        res = run_pinned(nc, group_in_maps(x, mu, gamma, b_per, cores), cores)
        for i, c in enumerate(cores):
            out[OFFSETS[c] : OFFSETS[c] + b_per] = res[i]["out"].astype(np.float32)
    return out


# revision 35
# speedup vs baseline: 1.0650x; 1.0650x over previous
"""ContextBasedLinear Trainium2 kernel.

Computes out = mu * x + gamma * sum(x, axis=1, keepdims=True) for
x: [64, 1024, 512] f32, mu/gamma: [1] f32.

Sharding: data-parallel on the batch dim across 8 NeuronCores;
mu/gamma replicated. No cross-core comms needed. The batch split is
UNEVEN: on this box the even-numbered devices are the only ones that
ever show a hot SDMA engine (~+18% DMA time; observed as DMA_0/DMA_15
across many runs, never on odd devices), so odd devices take 9 batches
and even devices 7 (64 total). Each batch count is its own Bass
program, launched on its device group with an explicit jax device list
(bass_utils' launcher always uses devices [0..k), which cannot target
the odd devices) — the HW exec metric is the max per-core useful span,
and this split equalizes the spans.

Per-core program (x_c: [b_per, 1024, 512]):
  Each batch's [1024, 512] lives in SBUF as [128, 4096]: partition p
  holds set rows 8p..8p+7 (16 KB contiguous per partition), processed
  in two half-tiles [128, 2048] for pipelining.
  - colsum: PE matmuls with ones[128,1] stationary reduce the
    partition dim of each 512-wide r-slice, accumulating all 8 slices
    into one PSUM row psum_s[1, 512]. Inputs viewed as float32r:
    single-pass fp32 matmul (1 cyc/row at N=512 vs 4 for the fp32
    LOW_HIGH split) — PE busy drops ~4x; the reduced-precision
    accumulate costs ~1e-3 rel err vs the 2e-2 gate. The BIR verifier
    wants FP32r operands produced "rounded": the x loads use f32r APs
    on both sides, and the ones lhsT is DMA-fed from a host input
    (walrus rejects f32r memset).
  - psum_b[128,512] = (gamma ones)[1,128].T @ s[1,512]: rank-1 matmul
    broadcasts gamma * colsum to every partition (plain fp32: only
    ~0.9us/batch of PE).
  - out = (x * mu) + psum_b in ONE fused DVE scalar_tensor_tensor pass
    per half, with psum_b read through a step-0 broadcast AP. The DVE
    writes fp16: stores move half the bytes (HBM-bound kernel; fp16
    rounding is ~5e-4 rel), upcast to f32 on the host.
  - Ring assignment: batch 0 loads BOTH halves on the sync HWDGE ring
    so its data completes first (~14 us); its colsum/bcast/first STT —
    and therefore the store stream — start early, which shrinks the
    end-of-kernel store drain. b1/b2 odd halves ride the (load-idle)
    ACT ring behind the tiny const DMAs; stores go on ACT except the
    last two batches' (sync), mirroring the drain.
"""

import numpy as np

import concourse.bacc as bacc
import concourse.mybir as mybir
import concourse.tile as tile

N_CORES = 8
B_FULL = 64
# batches per device (sum = 64). Launched as TWO sequential pinned
# launches split by device parity: under the all-8-concurrent single
# launch, 1-3 (always even-numbered) devices showed a hot SDMA engine
# (+18% DMA time); with the odd/even launches running back-to-back the
# hot engine has not reappeared, so the split stays uniform.
CORE_BATCHES = [8] * 8
OFFSETS = np.concatenate([[0], np.cumsum(CORE_BATCHES)])
GROUPS = []
for _cores in ([1, 3, 5, 7], [0, 2, 4, 6]):
    _bps = {CORE_BATCHES[c] for c in _cores}
    assert len(_bps) == 1
    GROUPS.append((_bps.pop(), list(_cores)))

N_SET = 1024
D = 512
P = 128
R = N_SET // P  # 8 set-rows per partition
F = R * D  # 4096 free elems per partition
H = 2  # half-tiles per batch
RH = R // H  # 4 r-slices per half
FH = F // H  # 2048 free elems per half

_cache = {}


def build_nc(b_per):
    if b_per in _cache:
        return _cache[b_per]
    f32 = mybir.dt.float32
    f32r = mybir.dt.float32r
    f16 = mybir.dt.float16
    nc = bacc.Bacc(
        "TRN2", target_bir_lowering=False, debug=False, num_devices=N_CORES
    )
    x_d = nc.dram_tensor("x", [b_per, N_SET, D], f32, kind="ExternalInput").ap()
    mu_d = nc.dram_tensor("mu", [1], f32, kind="ExternalInput").ap()
    gamma_d = nc.dram_tensor("gamma", [1], f32, kind="ExternalInput").ap()
    # host-fed ones: walrus can't memset f32r, but a DMA with f32r APs is
    # accepted as an FP32r-rounded producer for the colsum lhsT
    ones_d = nc.dram_tensor("ones", [P], f32, kind="ExternalInput").ap()
    out_d = nc.dram_tensor("out", [b_per, N_SET, D], f16, kind="ExternalOutput").ap()

    with tile.TileContext(nc) as tc:
        with (
            tc.tile_pool(name="consts", bufs=1) as consts,
            tc.tile_pool(name="xp", bufs=12) as xp,
            tc.tile_pool(name="op", bufs=9) as op,
            tc.tile_pool(name="sp", bufs=2) as sp,
            tc.tile_pool(name="ps", bufs=2, space="PSUM") as ps,
            tc.tile_pool(name="pb", bufs=2, space="PSUM") as pb,
        ):
            # ---- constants ----
            # The tiny const DMAs lead the ACT ring (ahead of the b1/b2 odd
            # halves): HWDGE delivers them by ~10.5 us. The SWDGE (gpsimd)
            # ring was tried and takes ~10 us to deliver 4 bytes — it gated
            # the first colsum until ~19 us.
            ones_col = consts.tile([P, 1], f32)  # colsum lhsT (K=128, M=1)
            nc.scalar.dma_start(
                ones_col[:].bitcast(f32r), ones_d[:, None].bitcast(f32r)
            )
            ones_row = consts.tile([1, P], f32)
            nc.vector.memset(ones_row, 1.0)
            mu_sb = consts.tile([1, 1], f32)
            nc.scalar.dma_start(mu_sb, mu_d[None, :])
            gamma_sb = consts.tile([1, 1], f32)
            nc.scalar.dma_start(gamma_sb, gamma_d[None, :])
            # gamma_row[1,128] = gamma * ones (runtime scalar from SBUF)
            gamma_row = consts.tile([1, P], f32)
            nc.vector.tensor_scalar_mul(gamma_row, ones_row, gamma_sb[:])
            # mu replicated to all 128 partitions via rank-1 matmul
            psum_mu = ps.tile([P, 1], f32, tag="psmu")
            nc.tensor.matmul(
                psum_mu, lhsT=ones_row[:], rhs=mu_sb[:], start=True, stop=True
            )
            mu_col = consts.tile([P, 1], f32)
            nc.vector.tensor_copy(mu_col, psum_mu)

            # ---- per-batch pipeline ----
            for b in range(b_per):
                x_view = x_d[b].rearrange("(p r) d -> p (r d)", p=P)
                o_view = out_d[b].rearrange("(p r) d -> p (r d)", p=P)

                xts = []
                for h in range(H):
                    xt = xp.tile([P, FH], f32, tag="xt")
                    # Batch 0 loads BOTH halves on the sync ring so its data
                    # completes first; b1/b2 odd halves ride the ACT ring to
                    # keep two HWDGE queue rows busy during the ramp (~390
                    # GB/s one row, ~422 with two); no HOL risk for those.
                    eng = nc.scalar if (b in (1, 2) and h == 1) else nc.sync
                    eng.dma_start(
                        xt[:].bitcast(f32r),
                        x_view[:, h * FH : (h + 1) * FH].bitcast(f32r),
                    )
                    xts.append(xt)

                # colsum over all 1024 set rows -> psum_s[1, 512]
                psum_s = ps.tile([1, D], f32, tag="pss")
                for h in range(H):
                    for j in range(RH):
                        nc.tensor.matmul(
                            psum_s,
                            lhsT=ones_col[:].bitcast(f32r),
                            rhs=xts[h][:, j * D : (j + 1) * D].bitcast(f32r),
                            start=(h == 0 and j == 0),
                            stop=(h == H - 1 and j == RH - 1),
                        )
                s_sb = sp.tile([1, D], f32, tag="ssb")
                nc.scalar.copy(s_sb, psum_s)

                # broadcast gamma*colsum to [128, 512] via rank-1 matmul
                psum_b = pb.tile([P, D], f32, tag="psb")
                nc.tensor.matmul(
                    psum_b, lhsT=gamma_row[:], rhs=s_sb[:], start=True, stop=True
                )

                # fused: out = (x * mu) + bcast   (single DVE pass per chunk).
                # The last batch runs quarter-size chunks so the kernel tail
                # (final STT + final store) is half as long.
                nq = 2 if b == b_per - 1 else 1
                fq = FH // nq
                rq = RH // nq
                for h in range(H):
                    for q in range(nq):
                        ot = op.tile([P, fq], f16, tag="ot")
                        nc.vector.scalar_tensor_tensor(
                            out=ot[:].rearrange("p (r d) -> p r d", r=rq),
                            in0=xts[h][:, q * fq : (q + 1) * fq].rearrange(
                                "p (r d) -> p r d", r=rq
                            ),
                            scalar=mu_col[:],
                            in1=psum_b[:, None, :].broadcast_to([P, rq, D]),
                            op0=mybir.AluOpType.mult,
                            op1=mybir.AluOpType.add,
                        )
                        # Mirror trick for the store-only drain: all loads
                        # have issued by the time the last batches store, so
                        # the sync row is free to carry half the tail stores.
                        seng = nc.sync if b >= b_per - 2 else nc.scalar
                        seng.dma_start(
                            o_view[:, h * FH + q * fq : h * FH + (q + 1) * fq], ot
                        )

    nc.compile()
    _cache[b_per] = nc
    return nc


def run_pinned(nc, in_maps, device_ids):
    """run_bass_via_pjrt with an explicit device list: lands a k-core
    launch on arbitrary physical cores (the stock launcher always takes
    jax.devices()[:k])."""
    import jax
    from jax.sharding import Mesh, PartitionSpec
    from jax.experimental.shard_map import shard_map

    from concourse import bass2jax
    from concourse.bass2jax import _bass_exec_p, partition_id_tensor

    bass2jax.install_neuronx_cc_hook()

    partition_name = nc.partition_id_tensor.name if nc.partition_id_tensor else None

    in_names, out_names, out_avals, zero_outs = [], [], [], []
    for alloc in nc.m.functions[0].allocations:
        if not isinstance(alloc, mybir.MemoryLocationSet):
            continue
        name = alloc.memorylocations[0].name
        if alloc.kind == "ExternalInput":
            if name != partition_name:
                in_names.append(name)
        elif alloc.kind == "ExternalOutput":
            shape = tuple(alloc.tensor_shape)
            dtype = mybir.dt.np(alloc.dtype)
            out_avals.append(jax.core.ShapedArray(shape, dtype))
            out_names.append(name)
            zero_outs.append(np.zeros(shape, dtype))
    n_params = len(in_names)
    n_outs = len(out_avals)
    in_names.extend(out_names)
    if partition_name is not None:
        in_names.append(partition_name)

    donate = tuple(range(n_params, n_params + n_outs))

    def _body(*args):
        operands = list(args)
        if partition_name is not None:
            operands.append(partition_id_tensor())
        outs = _bass_exec_p.bind(
            *operands,
            out_avals=tuple(out_avals),
            in_names=tuple(in_names),
            out_names=tuple(out_names),
            lowering_input_output_aliases=(),
            sim_require_finite=True,
            sim_require_nnan=True,
            nc=nc,
        )
        return tuple(outs)

    n_cores = len(device_ids)
    devices = [jax.devices()[i] for i in device_ids]
    mesh = Mesh(np.asarray(devices), ("core",))
    in_specs = (PartitionSpec("core"),) * (n_params + n_outs)
    out_specs = (PartitionSpec("core"),) * len(out_names)
    sharded = jax.jit(
        shard_map(
            _body, mesh=mesh, in_specs=in_specs, out_specs=out_specs, check_rep=False
        ),
        donate_argnums=donate,
        keep_unused=True,
    )
    per_core = [[np.asarray(m[name]) for name in in_names[:n_params]] for m in in_maps]
    concat_in = [
        np.concatenate([per_core[c][i] for c in range(n_cores)], axis=0)
        for i in range(n_params)
    ]
    concat_zeros = [
        np.zeros((n_cores * z.shape[0], *z.shape[1:]), z.dtype) for z in zero_outs
    ]
    out_arrs = sharded(*concat_in, *concat_zeros)
    return [
        {
            name: np.asarray(out_arrs[i]).reshape(n_cores, *out_avals[i].shape)[c]
            for i, name in enumerate(out_names)
        }
        for c in range(n_cores)
    ]


def group_in_maps(x, mu, gamma, b_per, cores):
    ones = np.ones([P], dtype=np.float32)
    return [
        {
            "x": x[OFFSETS[c] : OFFSETS[c] + b_per],
            "mu": mu,
            "gamma": gamma,
            "ones": ones,
        }
        for c in cores
    ]


def kernel(x, mu, gamma):
    x = np.ascontiguousarray(x, dtype=np.float32)
    mu = np.ascontiguousarray(mu, dtype=np.float32)
    gamma = np.ascontiguousarray(gamma, dtype=np.float32)
    out = np.empty((B_FULL, N_SET, D), dtype=np.float32)
    for b_per, cores in GROUPS:
        nc = build_nc(b_per)
        res = run_pinned(nc, group_in_maps(x, mu, gamma, b_per, cores), cores)
        for i, c in enumerate(cores):
            out[OFFSETS[c] : OFFSETS[c] + b_per] = res[i]["out"].astype(np.float32)
    return out
